# revision 23
# baseline (speedup 1.0000x reference)
"""BiMPM matching-layer kernel for 8 Trainium2 NeuronCores.

Data-parallel over the batch axis: each of the 8 cores gets 8 of the 64
batch elements (full hidden=100 and seq=384 on every core). Weights are
replicated (host-squared into the forms the device needs).

Per (batch, direction): m1 (cosine vs last q timestep), m2 (max pairwise
weighted cosine), m3 (cosine vs attention-mean of q), m4 (cosine vs
argmax-attended q), for l in {0,1}. The backward direction reuses the
*forward* w3/w4 tables (reference bug preserved). Outputs: 8 tensors of
shape (384, 64, 2), order m1f,m1b,m2f,m2b,m3f,m3b,m4f,m4b.

Precision plan: the m3 sign (1/sum of attention) and the m4 argmax are
ill-conditioned (gaps ~1e-4 / ~1e-6), so the p-q̂ attention matmul and
the q norms run in full fp32. Everything else (m2 numerators, hmean,
m4 dot products, p norms) tolerates bf16, which runs the PE at 1
cycle/row with fast weight loads instead of fp32's 4 cycles/row.
"""

import os
import sys

sys.path.insert(0, "/opt/trn_rl_repo")

import numpy as np

H, B, S, L = 100, 64, 384, 2
NCORES = 8
BC = B // NCORES  # 8 batches per core
NT = S // 128  # 3 tiles of 128 along seq
NPAIR = 2 * BC  # 16 (direction, batch) pairs per core

_COMPILED = {}


def _build_program():
    """Builds the single-core SPMD Bass program (same on all 8 cores)."""
    import concourse.bacc as bacc
    import concourse.bass as bass
    import concourse.mybir as mybir
    import concourse.tile as tile
    from concourse.bass_types import AP

    dt = mybir.dt
    f32 = dt.float32
    bf16 = dt.bfloat16
    AF = mybir.ActivationFunctionType
    ALU = mybir.AluOpType
    AX = mybir.AxisListType

    nc = bacc.Bacc("TRN2", target_bir_lowering=False, debug=False)

    ins = {}
    for nm in ("p_f", "p_b", "q_f", "q_b"):
        ins[nm] = nc.dram_tensor(nm, [H, BC, S], f32, kind="ExternalInput")
    WSB16 = nc.dram_tensor("WSB16", [H, 18], bf16, kind="ExternalInput")
    WSBF = nc.dram_tensor("WSBF", [H, 18], f32, kind="ExternalInput")
    U1C = nc.dram_tensor("U1C", [H, 4], f32, kind="ExternalInput")
    U2C = nc.dram_tensor("U2C", [H, 4], f32, kind="ExternalInput")
    UBC16 = nc.dram_tensor("UBC16", [128, 400], bf16, kind="ExternalInput")
    SEL16 = nc.dram_tensor("SEL16", [9, 384], bf16, kind="ExternalInput")
    IDN = nc.dram_tensor("IDN", [128, 128], f32, kind="ExternalInput")
    IDN16 = nc.dram_tensor("IDN16", [128, 128], bf16, kind="ExternalInput")
    outs = {}
    for nm in ("m1f", "m1b", "m2f", "m2b", "m3f", "m3b", "m4f", "m4b"):
        outs[nm] = nc.dram_tensor(nm, [S, BC, L], f32, kind="ExternalOutput")
    # per-pair DRAM scratch for the gather source (bf16 rows)
    GTs = [nc.dram_tensor(f"GT{i}", [S, 104], bf16) for i in range(NPAIR)]

    def ap3(t, off, pattern):
        """AP on tile t: partition dim + explicit free-dim [step,count]s."""
        base = t[:, 0:1]
        part = list(base.ap[0])
        return AP(base.tensor, base.offset + off,
                  [part] + [list(x) for x in pattern])

    with tile.TileContext(nc) as tc:
        con = tc.alloc_tile_pool(name="con", bufs=1)
        big = tc.alloc_tile_pool(name="big", bufs=2)
        sml = tc.alloc_tile_pool(name="sml", bufs=2)
        stg = tc.alloc_tile_pool(name="stg", bufs=1)
        ps = tc.alloc_tile_pool(name="ps", bufs=1, space="PSUM")

        # ---- constants -----------------------------------------------
        idn = con.tile([128, 128], f32, tag="idn")
        nc.sync.dma_start(idn[:], IDN.ap())
        idn16 = con.tile([128, 128], bf16, tag="idn16")
        nc.sync.dma_start(idn16[:], IDN16.ap())
        onesb = con.tile([128, 128], f32, tag="onesb")
        nc.vector.memset(onesb[0:1, :], 1.0)
        ones_row = onesb[0:1, :]
        wsb16_t = con.tile([128, 18], bf16, tag="wsb16")
        nc.sync.dma_start(wsb16_t[0:H, :], WSB16.ap())
        wsb16 = wsb16_t[0:H, :]
        wsbff_t = con.tile([128, 18], f32, tag="wsbff")
        nc.sync.dma_start(wsbff_t[0:H, :], WSBF.ap())
        wsbff = wsbff_t[0:H, :]
        u1c_t = con.tile([128, 4], f32, tag="u1c")
        nc.sync.dma_start(u1c_t[0:H, :], U1C.ap())
        u1c = u1c_t[0:H, :]
        u2c_t = con.tile([128, 4], f32, tag="u2c")
        nc.sync.dma_start(u2c_t[0:H, :], U2C.ap())
        u2c = u2c_t[0:H, :]
        ubc16 = con.tile([128, 400], bf16, tag="ubc16")
        nc.sync.dma_start(ubc16[:], UBC16.ap())
        sel_t = con.tile([128, 384], bf16, tag="sel16")
        nc.sync.dma_start(sel_t[0:9, :], SEL16.ap())
        sel16 = sel_t[0:9, :]
        zeros8 = con.tile([128, 8], f32, tag="zeros8")
        nc.vector.memset(zeros8[:], 0.0)

        # ---- persistent staging --------------------------------------
        # output staging: col = out_idx*48 + it*16 + b*2 + l
        stgt = stg.tile([128, 384], f32, tag="stgt")
        # per-pair scalars: col block pr*48 (layout matches old fsc)
        fscall = stg.tile([128, NPAIR * 48], f32, tag="fscall")
        # per-pair transposed inverse norms (27 cols each)
        ipnall = stg.tile([128, NPAIR * 27], f32, tag="ipnall")
        # per-pair [1/nq_0, 1/nq_1, 1, 1]
        nq4all = stg.tile([128, NPAIR * 4], f32, tag="nq4all")
        nc.vector.memset(ap3(nq4all, 2, [[4, NPAIR], [1, 2]]), 1.0)
        tmp34 = stg.tile([128, NPAIR * 12], f32, tag="tmp34")
        tmp12 = stg.tile([128, NPAIR * 12], f32, tag="tmp12")

        for di, d in enumerate(("f", "b")):
            P_in, Q_in = ins["p_" + d], ins["q_" + d]
            for b in range(BC):
                pr = di * BC + b
                GT = GTs[pr]

                def fscap(off, pattern):
                    return ap3(fscall, pr * 48 + off, pattern)

                fsc = fscall[:, pr * 48 : pr * 48 + 48]

                # ---- phase A: load, norms, scaled q variants ---------
                psbf_t = big.tile([128, S], f32, tag="psbf")
                qsbf_t = big.tile([128, S], f32, tag="qsbf")
                psbf, qsbf = psbf_t[0:H, :], qsbf_t[0:H, :]
                nc.sync.dma_start(psbf[:], P_in.ap()[:, b, :])
                nc.sync.dma_start(qsbf[:], Q_in.ap()[:, b, :])
                psb16_t = big.tile([128, S], bf16, tag="psb16")
                qsb16_t = big.tile([128, S], bf16, tag="qsb16")
                psq16_t = big.tile([128, S], bf16, tag="psq16")
                psb16, qsb16 = psb16_t[0:H, :], qsb16_t[0:H, :]
                psq16 = psq16_t[0:H, :]
                nc.scalar.activation(psb16[:], psbf[:], AF.Copy)
                nc.scalar.activation(qsb16[:], qsbf[:], AF.Copy)
                nc.scalar.activation(psq16[:], psbf[:], AF.Square)
                qsq_t = big.tile([128, S], f32, tag="qsq")
                qsq = qsq_t[0:H, :]
                nc.gpsimd.tensor_tensor(qsq[:], qsbf[:], qsbf[:],
                                        op=ALU.mult)

                pnqn = ps.tile([9, 896], f32, tag="pnqn")
                nc.tensor.matmul(pnqn[:, 0:S],
                                 wsb16[:, di * 9 : di * 9 + 9], psq16[:],
                                 start=True, stop=True)
                nc.tensor.matmul(pnqn[:, 512 : 512 + S],
                                 wsbff[:, di * 9 : di * 9 + 9], qsq[:],
                                 start=True, stop=True)
                ipqn_t = sml.tile([128, 2 * S], f32, tag="ipqn")
                rscr_t = sml.tile([128, 2 * S], f32, tag="rscr")
                ipqn, rscr = ipqn_t[0:9, :], rscr_t[0:9, :]
                from concourse.dve_ops import RECIPROCAL_APPROX_NR
                nc.scalar.activation(ap3(rscr, 0, [[S, 2], [1, S]]),
                                     ap3(pnqn, 0, [[512, 2], [1, S]]),
                                     AF.Sqrt)
                nc.vector.reciprocal_approx_fast(out=ipqn[:], in_=rscr[:])
                nc.vector._custom_dve(RECIPROCAL_APPROX_NR, out=ipqn[:],
                                      in0=rscr[:], in1=ipqn[:], s0=2.0)
                nc.vector._custom_dve(RECIPROCAL_APPROX_NR, out=ipqn[:],
                                      in0=rscr[:], in1=ipqn[:], s0=2.0)
                ipn = ipqn[:, 0:S]
                iqn = ipqn[:, S : 2 * S]
                # rsqrt Newton polish on the q rows: kills the ACT-LUT
                # sqrt error that perturbs argmax columns.
                nra = rscr[:, 0:S]
                nc.vector.tensor_tensor(nra[:], pnqn[:, 512 : 512 + S],
                                        iqn[:], op=ALU.mult)
                nc.vector.tensor_tensor(nra[:], nra[:], iqn[:], op=ALU.mult)
                nc.vector.tensor_scalar(nra[:], nra[:], -0.5, 1.5,
                                        op0=ALU.mult, op1=ALU.add)
                nc.vector.tensor_tensor(iqn[:], iqn[:], nra[:], op=ALU.mult)

                # misc PSUM bank: ipn transposed + m1 scale + iqn4 transp.
                misc = ps.tile([128, 512], f32, tag="misc")
                for it in range(NT):
                    nc.tensor.transpose(misc[:, it * 9 : it * 9 + 9],
                                        ipn[:, it * 128 : (it + 1) * 128],
                                        idn[:9, :9])
                # nq_l^2 = sum(u1_l * qlast^2) -> (1,2) at cols 32:34
                nc.tensor.matmul(misc[0:1, 32:34], qsq[:, S - 1 : S],
                                 u1c[:, di * 2 : di * 2 + 2],
                                 start=True, stop=True)
                nqr = sml.tile([128, 2], f32, tag="nqr")
                nc.scalar.activation(nqr[0:1, :], misc[0:1, 32:34], AF.Sqrt)
                nc.vector.reciprocal(nqr[0:1, :], nqr[0:1, :])
                nc.tensor.matmul(misc[:, 40:42], ones_row[:],
                                 nqr[0:1, :], start=True, stop=True)
                # inverse den4 columns: transpose iqn rows 0:3 per tile
                for it in range(NT):
                    nc.tensor.transpose(
                        misc[:, 64 + it * 3 : 64 + it * 3 + 3],
                        iqn[0:3, it * 128 : (it + 1) * 128],
                        idn[:3, :3])
                nc.vector.tensor_copy(ipnall[:, pr * 27 : pr * 27 + 27],
                                      misc[:, 0:27])
                nc.vector.tensor_copy(nq4all[:, pr * 4 : pr * 4 + 2],
                                      misc[:, 40:42])
                iq4t = sml.tile([128, 9], f32, tag="iq4t")
                nc.vector.tensor_copy(iq4t[:], misc[:, 64:73])

                # q variants: qn = q/|q| (fp32), q2n = q*u2_l/|w2_l q| (bf16)
                qn_t = big.tile([128, S + 2], f32, tag="qn")
                qn = qn_t[0:H, :]
                q2a_t = big.tile([128, 2 * S], bf16, tag="q2a")
                q2n_t = big.tile([128, 2 * S], bf16, tag="q2n")
                q2a, q2n = q2a_t[0:H, :], q2n_t[0:H, :]
                nc.scalar.activation(q2a[:, 0:S], qsbf[:], AF.Copy,
                                     scale=u2c[:, di * 2 : di * 2 + 1])
                nc.scalar.activation(q2a[:, S : 2 * S], qsbf[:], AF.Copy,
                                     scale=u2c[:, di * 2 + 1 : di * 2 + 2])
                iqr16_t = sml.tile([128, S], bf16, tag="iqr16")
                iqr16 = iqr16_t[0:5, :]
                nc.gpsimd.tensor_copy(iqr16[:], iqn[0:5, :])
                bcq = sml.tile([128, S], f32, tag="bcq")
                nc.gpsimd.partition_broadcast(bcq[0:H, :], iqn[0:1, :])
                nc.gpsimd.tensor_tensor(qn[:, 0:S], qsbf[:], bcq[0:H, :],
                                        op=ALU.mult)
                nc.gpsimd.tensor_tensor(
                    qn[:, S : S + 2],
                    qsbf[:, S - 1 : S].to_broadcast([H, 2]),
                    u1c[:, di * 2 : di * 2 + 2],
                    op=ALU.mult)
                for k in (1, 2):
                    bc = ps.tile([128, 384], f32, tag="ctt")
                    nc.tensor.matmul(
                        bc[0:H, 0:S],
                        sel16[0:5, k * 128 : k * 128 + H],
                        iqr16[:], start=True, stop=True)
                    off = (k - 1) * S
                    nc.vector.tensor_tensor(q2n[:, off : off + S],
                                            q2a[:, off : off + S],
                                            bc[0:H, 0:S], op=ALU.mult)

                # ---- phase B: i-layout matmuls + fused maxes ---------
                idx8 = [sml.tile([128, 8], dt.uint32, tag=f"idx{it}",
                                 name=f"idx{it}")
                        for it in range(NT)]
                att16 = big.tile([128, S], bf16, tag="att16")
                cts16 = big.tile([128, NT * S], bf16, tag="cts16")
                for it in range(NT):
                    mega = ps.tile([128, 1536], f32, tag="mega")
                    lhs16 = psb16[:, it * 128 : (it + 1) * 128]
                    nc.tensor.matmul(mega[:, 0:S], lhs16,
                                     q2n[:, 0:S], start=True, stop=True)
                    nc.tensor.matmul(mega[:, 512 : 512 + S], lhs16,
                                     q2n[:, S : 2 * S],
                                     start=True, stop=True)
                    nc.tensor.matmul(
                        mega[:, 1024 : 1024 + S + 2],
                        psbf[:, it * 128 : (it + 1) * 128],
                        qn[:], start=True, stop=True)
                    nc.vector.reduce_max(
                        fsc[:, it * 16 + 8 : it * 16 + 11],
                        ap3(mega, 0, [[512, 3], [1, S]]), axis=AX.X)
                    mx8 = sml.tile([128, 8], f32, tag="mx8")
                    nc.vector.tensor_scalar(
                        mx8[:], zeros8[:],
                        fsc[:, it * 16 + 10 : it * 16 + 11], None,
                        op0=ALU.add)
                    nc.vector.max_index(
                        idx8[it][:], mx8[:],
                        mega[:, 1024 : 1024 + S])
                    nc.vector.tensor_copy(
                        fsc[:, it * 16 + 11 : it * 16 + 13],
                        mega[:, 1024 + S : 1024 + S + 2])
                    # row copy in bf16 (reused as hmean lhsT) + row sum
                    # for the m3 attention denominator -- on ScalarE.
                    nc.scalar.activation(
                        att16[:], mega[:, 1024 : 1024 + S], AF.Copy,
                        accum_out=fsc[:, it * 16 + 14 : it * 16 + 15])
                    # transpose p.q-hat into j-major layout for hmean
                    ctt = ps.tile([128, 384], bf16, tag="ctt")
                    for jt in range(NT):
                        nc.tensor.transpose(
                            ctt[:, jt * 128 : (jt + 1) * 128],
                            att16[:, jt * 128 : (jt + 1) * 128],
                            idn16[:, :])
                    nc.vector.tensor_copy(
                        ap3(cts16, it * 128, [[S, NT], [1, 128]]),
                        ctt[:])

                # ---- phase C: T-layout rhs packs (batched over jt) ---
                hmrhs = sml.tile([128, 3 * 304], bf16, tag="hmrhs")
                ptsa = sml.tile([128, 600], bf16, tag="ptsa")
                p4ua = sml.tile([128, 600], bf16, tag="p4ua")
                gsrc = sml.tile([128, 312], bf16, tag="gsrc")
                tqp = ps.tile([128, 672], bf16, tag="misc")
                for jt in range(NT):
                    nc.tensor.transpose(tqp[:, jt * 224 : jt * 224 + H],
                                        qsb16[:, jt * 128 : (jt + 1) * 128],
                                        idn16[:H, :H])
                    nc.tensor.transpose(
                        tqp[:, jt * 224 + 112 : jt * 224 + 112 + H],
                        psb16[:, jt * 128 : (jt + 1) * 128],
                        idn16[:H, :H])
                nc.vector.tensor_copy(ap3(hmrhs, 0, [[304, 3], [1, H]]),
                                      ap3(tqp, 0, [[224, 3], [1, H]]))
                # q3uT_l = qT * u3_l for l=0,1 (read qT twice per jt)
                nc.vector.tensor_tensor(
                    ap3(hmrhs, H, [[304, 3], [1, 2 * H]]),
                    ap3(hmrhs, 0, [[304, 3], [0, 2], [1, H]]),
                    ap3(ubc16, 0, [[0, 3], [1, 2 * H]]), op=ALU.mult)
                nc.vector.tensor_copy(ap3(gsrc, 0, [[104, 3], [1, H]]),
                                      ap3(tqp, 0, [[224, 3], [1, H]]))
                nc.vector.tensor_copy(ap3(gsrc, H, [[104, 3], [1, 2]]),
                                      ap3(iq4t, 1, [[3, 3], [1, 2]]))
                nc.vector.memset(ap3(gsrc, H + 2, [[104, 3], [1, 2]]), 0.0)
                nc.sync.dma_start(
                    AP(GT, 0, [[104, 128], [128 * 104, 3], [1, 104]]),
                    ap3(gsrc, 0, [[104, 3], [1, 104]]))
                nc.vector.tensor_copy(ap3(ptsa, 0, [[200, 3], [1, H]]),
                                      ap3(tqp, 112, [[224, 3], [1, H]]))
                # p4uT pack = [pT*u4_0 | pT*u4_1] per it
                nc.gpsimd.tensor_tensor(
                    p4ua[:],
                    ap3(ptsa, 0, [[200, 3], [0, 2], [1, H]]),
                    ap3(ubc16, 2 * H, [[0, 3], [1, 2 * H]]), op=ALU.mult)

                # ---- phase D: hm matmuls, gather, dot products -------
                # m3 = sign(sum_att) * n3_raw / sqrt(d3_raw): the 1/sum
                # magnitude cancels in the cosine, only its sign matters
                # (applied in phase E), so no reciprocal is needed here.
                pscr = sml.tile([128, 1200], f32, tag="pscr")
                gall = sml.tile([128, 312], bf16, tag="gout")
                hmxa = ps.tile([128, 1536], f32, tag="mega")
                for it in range(NT):
                    for jt in range(NT):
                        nc.tensor.matmul(
                            hmxa[:, it * 512 : it * 512 + 300],
                            cts16[:, jt * S + it * 128
                                  : jt * S + (it + 1) * 128],
                            hmrhs[:, jt * 304 : jt * 304 + 300],
                            start=(jt == 0), stop=(jt == NT - 1))
                    nc.gpsimd.indirect_dma_start(
                        out=gall[:, it * 104 : (it + 1) * 104],
                        out_offset=None, in_=GT.ap(),
                        in_offset=bass.IndirectOffsetOnAxis(
                            ap=idx8[it][:, 0:1], axis=0))
                # hm (raw) -> next to pT for the m3 d3 products
                nc.vector.tensor_copy(ap3(ptsa, H, [[200, 3], [1, H]]),
                                      ap3(hmxa, 0, [[512, 3], [1, H]]))
                # m3 numerator/denominator products, all its at once
                nc.vector.tensor_tensor(
                    ap3(pscr, 0, [[400, 3], [1, 2 * H]]),
                    ap3(hmxa, H, [[512, 3], [1, 2 * H]]),
                    ap3(ptsa, 0, [[200, 3], [0, 2], [1, H]]), op=ALU.mult)
                nc.vector.tensor_tensor(
                    ap3(pscr, 2 * H, [[400, 3], [1, 2 * H]]),
                    ap3(hmxa, H, [[512, 3], [1, 2 * H]]),
                    ap3(ptsa, H, [[200, 3], [0, 2], [1, H]]), op=ALU.mult)
                nc.vector.reduce_sum(
                    fscap(0, [[16, 3], [1, 2]]),
                    ap3(pscr, 0, [[400, 3], [100, 2], [1, H]]), axis=AX.X)
                nc.vector.reduce_sum(
                    fscap(4, [[16, 3], [1, 2]]),
                    ap3(pscr, 2 * H, [[400, 3], [100, 2], [1, H]]),
                    axis=AX.X)
                # m4 products: hmaxT (read twice) * [p4uT_0 | p4uT_1]
                nc.gpsimd.tensor_tensor(
                    pscr[:, 0:600],
                    ap3(gall, 0, [[104, 3], [0, 2], [1, H]]),
                    p4ua[:], op=ALU.mult)
                nc.vector.reduce_sum(
                    fscap(2, [[16, 3], [1, 2]]),
                    ap3(pscr, 0, [[200, 3], [100, 2], [1, H]]), axis=AX.X)
                nc.gpsimd.tensor_copy(
                    fscap(6, [[16, 3], [1, 2]]),
                    ap3(gall, H, [[104, 3], [1, 2]]))

        # ---- phase E: finalize all pairs at once ---------------------
        # sign(sum_att) for m3 (the 1/sum magnitude cancels in the cosine)
        nc.scalar.activation(
            ap3(fscall, 13, [[48, NPAIR], [16, 3], [1, 1]]),
            ap3(fscall, 14, [[48, NPAIR], [16, 3], [1, 1]]), AF.Sign)
        # 1/sqrt over the m3/m4 denominators (cols +4..+6 per it block)
        den = ap3(fscall, 4, [[48, NPAIR], [16, 3], [1, 2]])
        nc.scalar.activation(den, den, AF.Sqrt)
        nc.vector.reciprocal(den, den)
        # m3/m4: tmp34 = num * invden, then * transposed 1/|w p| norms
        nc.vector.tensor_tensor(
            tmp34[:], ap3(fscall, 0, [[48, NPAIR], [16, 3], [1, 4]]),
            ap3(fscall, 4, [[48, NPAIR], [16, 3], [1, 4]]), op=ALU.mult)
        nc.vector.tensor_tensor(
            ap3(tmp34, 0, [[12, NPAIR], [4, 3], [1, 2]]),
            ap3(tmp34, 0, [[12, NPAIR], [4, 3], [1, 2]]),
            ap3(fscall, 13, [[48, NPAIR], [16, 3], [0, 2]]), op=ALU.mult)
        for di in range(2):
            for o in range(2):  # o=0: m3, o=1: m4
                nc.vector.tensor_tensor(
                    ap3(stgt, (4 + di) * 48 + 96 * o,
                        [[2, BC], [16, 3], [1, 2]]),
                    ap3(tmp34, di * BC * 12 + o * 2,
                        [[12, BC], [4, 3], [1, 2]]),
                    ap3(ipnall, di * BC * 27 + (7 if o == 0 else 1),
                        [[27, BC], [9, 3], [1, 2]]),
                    op=ALU.mult)
        # m1/m2: tmp12 = num * 1/|w p|, then m1 *= 1/nq
        for di in range(2):
            for o in range(2):  # o=0: m1, o=1: m2
                nc.vector.tensor_tensor(
                    ap3(tmp12, di * BC * 12 + o * 2,
                        [[12, BC], [4, 3], [1, 2]]),
                    ap3(fscall, di * BC * 48 + (11 if o == 0 else 8),
                        [[48, BC], [16, 3], [1, 2]]),
                    ap3(ipnall, di * BC * 27 + (5 if o == 0 else 3),
                        [[27, BC], [9, 3], [1, 2]]),
                    op=ALU.mult)
                nc.vector.tensor_tensor(
                    ap3(stgt, di * 48 + 96 * o, [[2, BC], [16, 3], [1, 2]]),
                    ap3(tmp12, di * BC * 12 + o * 2,
                        [[12, BC], [4, 3], [1, 2]]),
                    ap3(nq4all, di * BC * 4 + (0 if o == 0 else 2),
                        [[4, BC], [0, 3], [1, 2]]),
                    op=ALU.mult)

        for oi, nm in enumerate(("m1f", "m1b", "m2f", "m2b",
                                 "m3f", "m3b", "m4f", "m4b")):
            t = outs[nm]
            out_ap = AP(t, 0, [[16, 128], [2048, 3], [1, 16]])
            in_ap = ap3(stgt, oi * 48, [[16, 3], [1, 16]])
            nc.sync.dma_start(out_ap, in_ap)

        for p in (ps, stg, sml, big, con):
            p.release()

    nc.compile()
    return nc


def _host_consts(w1, w2, w3f_, w4f_):
    u1 = (w1 * w1).astype(np.float32)
    u2 = (w2 * w2).astype(np.float32)
    u3 = (w3f_ * w3f_).astype(np.float32)
    u4 = (w4f_ * w4f_).astype(np.float32)
    ones = np.ones((H,), np.float32)
    wsb = np.stack([ones, u4[0], u4[1], u2[0], u2[1],
                    u1[0], u1[1], u3[0], u3[1]], axis=1)  # (H, 9)
    return u1, u2, u3, u4, wsb


def _prepare(p_f, p_b, q_f, q_b, w1f, w1b, w2f, w2b, w3f, w3b, w4f, w4b):
    import ml_dtypes

    bfd = ml_dtypes.bfloat16
    if "prog" not in _COMPILED:
        _COMPILED["prog"] = _build_program()
    nc = _COMPILED["prog"]

    p_f, p_b = np.asarray(p_f), np.asarray(p_b)
    q_f, q_b = np.asarray(q_f), np.asarray(q_b)
    u1f, u2f_, u3, u4, wsbf = _host_consts(
        np.asarray(w1f), np.asarray(w2f), np.asarray(w3f), np.asarray(w4f))
    u1b, u2b_, _, _, wsbb = _host_consts(
        np.asarray(w1b), np.asarray(w2b), np.asarray(w3f), np.asarray(w4f))
    WSB = np.concatenate([wsbf, wsbb], axis=1).astype(np.float32)
    U1C = np.stack([u1f[0], u1f[1], u1b[0], u1b[1]], 1).astype(np.float32)
    U2C = np.stack([u2f_[0], u2f_[1], u2b_[0], u2b_[1]], 1).astype(np.float32)
    ubc = np.concatenate([u3[0], u3[1], u4[0], u4[1]]).astype(np.float32)
    UBC16 = np.ascontiguousarray(
        np.broadcast_to(ubc, (128, 400))).astype(bfd)
    SEL16 = np.zeros((9, 384), np.float32)
    SEL16[3, 128:256] = 1.0  # invq2_0
    SEL16[4, 256:384] = 1.0  # invq2_1
    IDN = np.eye(128, dtype=np.float32)

    in_maps = []
    for c in range(NCORES):
        sl = slice(c * BC, (c + 1) * BC)
        in_maps.append({
            "p_f": np.ascontiguousarray(p_f[:, sl, :]),
            "p_b": np.ascontiguousarray(p_b[:, sl, :]),
            "q_f": np.ascontiguousarray(q_f[:, sl, :]),
            "q_b": np.ascontiguousarray(q_b[:, sl, :]),
            "WSB16": WSB.astype(bfd), "WSBF": WSB,
            "U1C": U1C, "U2C": U2C, "UBC16": UBC16,
            "SEL16": SEL16.astype(bfd),
            "IDN": IDN, "IDN16": IDN.astype(bfd),
        })

    return nc, in_maps


def _gather_outputs(results):
    full = []
    for nm in ("m1f", "m1b", "m2f", "m2b", "m3f", "m3b", "m4f", "m4b"):
        full.append(np.concatenate([results[c][nm] for c in range(NCORES)],
                                   axis=1))
    return tuple(full)


def kernel(**inputs):
    from concourse.bass_utils import run_bass_kernel_spmd

    nc, in_maps = _prepare(**inputs)
    res = run_bass_kernel_spmd(nc, in_maps, list(range(NCORES)))
    return _gather_outputs(res.results)


def run_traced(**inputs):
    """Run with NTFF profiling; returns (exec_time_ns, results_obj)."""
    from concourse.bass_utils import run_bass_kernel_spmd

    nc, in_maps = _prepare(**inputs)
    res = run_bass_kernel_spmd(nc, in_maps, list(range(NCORES)), trace=True)
    return res.exec_time_ns, res


# revision 51
# speedup vs baseline: 1.6684x; 1.6684x over previous
"""BiMPM matching-layer kernel for 8 Trainium2 NeuronCores.

Data-parallel over the batch axis: each of the 8 cores gets 8 of the 64
batch elements (full hidden=100 and seq=384 on every core). Weights are
replicated (host-squared into the forms the device needs).

Per (batch, direction): m1 (cosine vs last q timestep), m2 (max pairwise
weighted cosine), m3 (cosine vs attention-mean of q), m4 (cosine vs
argmax-attended q), for l in {0,1}. The backward direction reuses the
*forward* w3/w4 tables (reference bug preserved). Outputs: 8 tensors of
shape (384, 64, 2), order m1f,m1b,m2f,m2b,m3f,m3b,m4f,m4b.

Precision plan: the m3 sign (1/sum of attention) and the m4 argmax are
ill-conditioned (gaps ~1e-4 / ~1e-6), so the p-q̂ attention matmul and
the q norms run in full fp32. Everything else (m2 numerators, hmean,
m4 dot products, p norms) tolerates bf16, which runs the PE at 1
cycle/row with fast weight loads instead of fp32's 4 cycles/row.
"""

import os
import sys

sys.path.insert(0, "/opt/trn_rl_repo")

import numpy as np

H, B, S, L = 100, 64, 384, 2
NCORES = 8
BC = B // NCORES  # 8 batches per core
NT = S // 128  # 3 tiles of 128 along seq
NPAIR = 2 * BC  # 16 (direction, batch) pairs per core

_COMPILED = {}


def _build_program():
    """Builds the single-core SPMD Bass program (same on all 8 cores)."""
    import concourse.bacc as bacc
    import concourse.bass as bass
    import concourse.mybir as mybir
    import concourse.tile as tile
    from concourse.bass_types import AP

    dt = mybir.dt
    f32 = dt.float32
    bf16 = dt.bfloat16
    AF = mybir.ActivationFunctionType
    ALU = mybir.AluOpType
    AX = mybir.AxisListType

    nc = bacc.Bacc("TRN2", target_bir_lowering=False, debug=False)

    ins = {}
    for nm in ("p_f", "p_b", "q_f", "q_b"):
        ins[nm] = nc.dram_tensor(nm, [H, BC, S], f32, kind="ExternalInput")
    WSB16 = nc.dram_tensor("WSB16", [H, 18], bf16, kind="ExternalInput")
    WSBF = nc.dram_tensor("WSBF", [H, 18], f32, kind="ExternalInput")
    U1C = nc.dram_tensor("U1C", [H, 4], f32, kind="ExternalInput")
    U2C = nc.dram_tensor("U2C", [H, 4], f32, kind="ExternalInput")
    UBC16 = nc.dram_tensor("UBC16", [128, 400], bf16, kind="ExternalInput")
    SEL16 = nc.dram_tensor("SEL16", [9, 384], bf16, kind="ExternalInput")
    IDN = nc.dram_tensor("IDN", [128, 128], f32, kind="ExternalInput")
    IDN16 = nc.dram_tensor("IDN16", [128, 128], bf16, kind="ExternalInput")
    outs = {}
    for nm in ("m1f", "m1b", "m2f", "m2b", "m3f", "m3b", "m4f", "m4b"):
        outs[nm] = nc.dram_tensor(nm, [S, BC, L], f32, kind="ExternalOutput")
    # per-pair DRAM scratch for the gather source (bf16 rows)
    GTs = [nc.dram_tensor(f"GT{i}", [S, 104], bf16) for i in range(NPAIR)]

    def ap3(t, off, pattern):
        """AP on tile t: partition dim + explicit free-dim [step,count]s."""
        base = t[:, 0:1]
        part = list(base.ap[0])
        return AP(base.tensor, base.offset + off,
                  [part] + [list(x) for x in pattern])

    with tile.TileContext(nc) as tc:
        con = tc.alloc_tile_pool(name="con", bufs=1)
        big = tc.alloc_tile_pool(name="big", bufs=3)
        sml = tc.alloc_tile_pool(name="sml", bufs=3)
        stg = tc.alloc_tile_pool(name="stg", bufs=1)
        ps = tc.alloc_tile_pool(name="ps", bufs=1, space="PSUM")
        psm = tc.alloc_tile_pool(name="psm", bufs=2, space="PSUM")

        # ---- constants -----------------------------------------------
        idn = con.tile([128, 128], f32, tag="idn")
        nc.sync.dma_start(idn[:], IDN.ap())
        idn16 = con.tile([128, 128], bf16, tag="idn16")
        nc.sync.dma_start(idn16[:], IDN16.ap())
        onesb = con.tile([128, 128], f32, tag="onesb")
        nc.vector.memset(onesb[0:1, :], 1.0)
        ones_row = onesb[0:1, :]
        wsb16_t = con.tile([128, 18], bf16, tag="wsb16")
        nc.sync.dma_start(wsb16_t[0:H, :], WSB16.ap())
        wsb16 = wsb16_t[0:H, :]
        wsbff_t = con.tile([128, 18], f32, tag="wsbff")
        nc.sync.dma_start(wsbff_t[0:H, :], WSBF.ap())
        wsbff = wsbff_t[0:H, :]
        u1c_t = con.tile([128, 4], f32, tag="u1c")
        nc.sync.dma_start(u1c_t[0:H, :], U1C.ap())
        u1c = u1c_t[0:H, :]
        u2c_t = con.tile([128, 4], f32, tag="u2c")
        nc.sync.dma_start(u2c_t[0:H, :], U2C.ap())
        u2c = u2c_t[0:H, :]
        ubc16 = con.tile([128, 400], bf16, tag="ubc16")
        nc.sync.dma_start(ubc16[:], UBC16.ap())
        sel_t = con.tile([128, 384], bf16, tag="sel16")
        nc.sync.dma_start(sel_t[0:9, :], SEL16.ap())
        sel16 = sel_t[0:9, :]
        zeros8 = con.tile([128, 8], f32, tag="zeros8")
        nc.vector.memset(zeros8[:], 0.0)

        # ---- persistent staging --------------------------------------
        # output staging: col = out_idx*48 + it*16 + b*2 + l
        stgt = stg.tile([128, 384], f32, tag="stgt")
        # per-pair scalars: col block pr*48 (layout matches old fsc)
        fscall = stg.tile([128, NPAIR * 48], f32, tag="fscall")
        # per-pair transposed inverse norms (27 cols each)
        ipnall = stg.tile([128, NPAIR * 27], f32, tag="ipnall")
        # per-pair [1/nq_0, 1/nq_1, 1, 1]
        nq4all = stg.tile([128, NPAIR * 4], f32, tag="nq4all")
        nc.vector.memset(ap3(nq4all, 2, [[4, NPAIR], [1, 2]]), 1.0)
        tmp34 = stg.tile([128, NPAIR * 12], f32, tag="tmp34")
        tmp12 = stg.tile([128, NPAIR * 12], f32, tag="tmp12")
        # p/q fp32 loads persist across pass 1 and pass 2
        pq = tc.alloc_tile_pool(name="pq", bufs=BC + 1)
        # batched-norm workspace (4 pairs at a time, double-buffered)
        nrm = tc.alloc_tile_pool(name="nrm", bufs=2)
        NSUB = 2
        QB = BC // NSUB

        from concourse.dve_ops import RECIPROCAL_APPROX_NR

        for di, d in enumerate(("f", "b")):
          P_in, Q_in = ins["p_" + d], ins["q_" + d]
          for sub in range(NSUB):
            b0 = sub * QB
            pnall = nrm.tile([128, QB * 768], f32, tag="pnall")
            ipqa = nrm.tile([128, QB * 768], f32, tag="ipqa")
            rsca = nrm.tile([128, QB * 768], f32, tag="rsca")
            iqr16a = nrm.tile([128, QB * 384], bf16, tag="iqr16a")
            if True:
            # ---- pass 1: loads, squares, norm matmuls ----------------
              pqs = []
              for b in range(b0, b0 + QB):
                pr = di * BC + b
                psbf_t = pq.tile([128, S], f32, tag="psbf")
                qsbf_t = pq.tile([128, S], f32, tag="qsbf")
                psbf, qsbf = psbf_t[0:H, :], qsbf_t[0:H, :]
                pqs.append((psbf, qsbf))
                nc.sync.dma_start(psbf[:], P_in.ap()[:, b, :])
                nc.sync.dma_start(qsbf[:], Q_in.ap()[:, b, :])
                psq16_t = big.tile([128, S], bf16, tag="psq16")
                psq16 = psq16_t[0:H, :]
                nc.scalar.activation(psq16[:], psbf[:], AF.Square)
                qsq_t = big.tile([128, S], f32, tag="qsq")
                qsq = qsq_t[0:H, :]
                nc.gpsimd.tensor_tensor(qsq[:], qsbf[:], qsbf[:],
                                        op=ALU.mult)
                pnqn = psm.tile([9, 1536], f32, tag="mega")
                nc.tensor.matmul(pnqn[:, 0:S],
                                 wsb16[:, di * 9 : di * 9 + 9], psq16[:],
                                 start=True, stop=True)
                nc.tensor.matmul(pnqn[:, 512 : 512 + S],
                                 wsbff[:, di * 9 : di * 9 + 9], qsq[:],
                                 start=True, stop=True)
                nc.scalar.activation(
                    pnall[0:9, (b - b0) * 768 : (b - b0 + 1) * 768],
                    ap3(pnqn, 0, [[512, 2], [1, S]]), AF.Copy)
                # m1 q-side norm: nq^2 = sum(u1_l * qlast^2)
                misc = ps.tile([128, 512], f32, tag="misc")
                nc.tensor.matmul(misc[0:1, 32:34], qsq[:, S - 1 : S],
                                 u1c[:, di * 2 : di * 2 + 2],
                                 start=True, stop=True)
                nqr = sml.tile([128, 2], f32, tag="nqr")
                nc.scalar.activation(nqr[0:1, :], misc[0:1, 32:34], AF.Sqrt)
                nc.vector.reciprocal(nqr[0:1, :], nqr[0:1, :])
                nc.tensor.matmul(misc[:, 40:42], ones_row[:],
                                 nqr[0:1, :], start=True, stop=True)
                nc.vector.tensor_copy(nq4all[:, pr * 4 : pr * 4 + 2],
                                      misc[:, 40:42])

            # ---- batched 1/sqrt pipeline over the 4 pairs ------------
            pn9, ip9, rs9 = pnall[0:9, :], ipqa[0:9, :], rsca[0:9, :]
            nc.scalar.activation(rs9[:], pn9[:], AF.Sqrt)
            nc.vector.reciprocal_approx_fast(out=ip9[:], in_=rs9[:])
            # only iqn row 0 (1/|q|, argmax-critical) needs more than the
            # ~3e-4 LUT+approx accuracy: NR the reciprocal, then one
            # rsqrt Newton polish against the exact norm^2.
            ip0 = ap3(ipqa[0:1, :], S, [[768, QB], [1, S]])
            rs0 = ap3(rsca[0:1, :], S, [[768, QB], [1, S]])
            nc.vector._custom_dve(RECIPROCAL_APPROX_NR, out=ip0,
                                  in0=rs0, in1=ip0, s0=2.0)
            nc.vector._custom_dve(RECIPROCAL_APPROX_NR, out=ip0,
                                  in0=rs0, in1=ip0, s0=2.0)
            nraap = ap3(rsca[0:1, :], S, [[768, QB], [1, S]])
            pnq = ap3(pnall[0:1, :], S, [[768, QB], [1, S]])
            nc.vector.tensor_tensor(nraap, pnq, ip0, op=ALU.mult)
            nc.vector.tensor_tensor(nraap, nraap, ip0, op=ALU.mult)
            nc.vector.tensor_scalar(nraap, nraap, -0.5, 1.5,
                                    op0=ALU.mult, op1=ALU.add)
            nc.vector.tensor_tensor(ip0, ip0, nraap, op=ALU.mult)
            nc.vector.tensor_copy(iqr16a[0:5, :],
                                  ap3(ipqa[0:5, :], S, [[768, QB], [1, S]]))

            # ---- pass 2: per-pair compute ----------------------------
            if True:
              for b in range(b0, b0 + QB):
                pr = di * BC + b
                bl = b - b0
                GT = GTs[pr]

                def fscap(off, pattern):
                    return ap3(fscall, pr * 48 + off, pattern)

                fsc = fscall[:, pr * 48 : pr * 48 + 48]
                ipn = ipqa[0:9, bl * 768 : bl * 768 + S]
                iqn = ipqa[0:9, bl * 768 + S : bl * 768 + 2 * S]
                iqr16 = iqr16a[0:5, bl * 384 : (bl + 1) * 384]

                psbf, qsbf = pqs[bl]
                psb16_t = big.tile([128, S], bf16, tag="psb16")
                qsb16_t = big.tile([128, S], bf16, tag="qsb16")
                psb16, qsb16 = psb16_t[0:H, :], qsb16_t[0:H, :]
                nc.scalar.activation(psb16[:], psbf[:], AF.Copy)
                nc.scalar.activation(qsb16[:], qsbf[:], AF.Copy)

                # transposed inverse norms
                misc = ps.tile([128, 512], f32, tag="misc")
                for it in range(NT):
                    nc.tensor.transpose(misc[:, it * 9 : it * 9 + 9],
                                        ipn[:, it * 128 : (it + 1) * 128],
                                        idn[:9, :9])
                for it in range(NT):
                    nc.tensor.transpose(
                        misc[:, 64 + it * 3 : 64 + it * 3 + 3],
                        iqn[0:3, it * 128 : (it + 1) * 128],
                        idn[:3, :3])
                nc.scalar.activation(ipnall[:, pr * 27 : pr * 27 + 27],
                                     misc[:, 0:27], AF.Copy)
                iq4t = sml.tile([128, 9], f32, tag="iq4t")
                nc.scalar.activation(iq4t[:], misc[:, 64:73], AF.Copy)

                # q variants: qn = q/|q| (fp32), q2n = q*u2_l/|w2_l q| (bf16)
                qn_t = big.tile([128, S + 4], f32, tag="qn")
                qn = qn_t[0:H, :]
                q2a_t = big.tile([128, 2 * S], bf16, tag="q2a")
                q2n_t = big.tile([128, 2 * S], bf16, tag="q2n")
                q2a, q2n = q2a_t[0:H, :], q2n_t[0:H, :]
                nc.scalar.activation(q2a[:, 0:S], qsbf[:], AF.Copy,
                                     scale=u2c[:, di * 2 : di * 2 + 1])
                nc.scalar.activation(q2a[:, S : 2 * S], qsbf[:], AF.Copy,
                                     scale=u2c[:, di * 2 + 1 : di * 2 + 2])
                bcq = sml.tile([128, S], f32, tag="bcq")
                nc.gpsimd.partition_broadcast(bcq[0:H, :], iqn[0:1, :])
                nc.vector.tensor_tensor(qn[:, 0:S], qsbf[:], bcq[0:H, :],
                                        op=ALU.mult)
                nc.vector.tensor_tensor(
                    qn[:, S : S + 2],
                    qsbf[:, S - 1 : S].to_broadcast([H, 2]),
                    u1c[:, di * 2 : di * 2 + 2],
                    op=ALU.mult)

                for k in (1, 2):
                    bc = ps.tile([128, 384], f32, tag="ctt")
                    nc.tensor.matmul(
                        bc[0:H, 0:S],
                        sel16[0:5, k * 128 : k * 128 + H],
                        iqr16[:], start=True, stop=True)
                    off = (k - 1) * S
                    nc.vector.tensor_tensor(q2n[:, off : off + S],
                                            q2a[:, off : off + S],
                                            bc[0:H, 0:S], op=ALU.mult)

                # ---- phase B: i-layout matmuls + fused maxes ---------
                idx8 = [sml.tile([128, 8], dt.uint32, tag=f"idx{it}",
                                 name=f"idx{it}")
                        for it in range(NT)]
                cts16 = big.tile([128, NT * S], bf16, tag="cts16")
                for it in range(NT):
                    mega = psm.tile([128, 1536], f32, tag="mega")
                    lhs16 = psb16[:, it * 128 : (it + 1) * 128]
                    nc.tensor.matmul(mega[:, 0:S], lhs16,
                                     q2n[:, 0:S], start=True, stop=True)
                    nc.tensor.matmul(mega[:, 512 : 512 + S], lhs16,
                                     q2n[:, S : 2 * S],
                                     start=True, stop=True)
                    nc.tensor.matmul(
                        mega[:, 1024 : 1024 + S + 2],
                        psbf[:, it * 128 : (it + 1) * 128],
                        qn[:, 0 : S + 2], start=True, stop=True)
                    nc.vector.reduce_max(
                        fsc[:, it * 16 + 8 : it * 16 + 11],
                        ap3(mega, 0, [[512, 3], [1, S]]), axis=AX.X)
                    nc.vector.max_index(
                        idx8[it][:], fscap(it * 16 + 10, [[0, 8]]),
                        mega[:, 1024 : 1024 + S])
                    nc.scalar.activation(
                        fsc[:, it * 16 + 11 : it * 16 + 13],
                        mega[:, 1024 + S : 1024 + S + 2], AF.Copy)
                    att16 = big.tile([128, S], bf16, tag="att16")
                    nc.scalar.activation(
                        att16[:], mega[:, 1024 : 1024 + S], AF.Copy,
                        accum_out=fsc[:, it * 16 + 14 : it * 16 + 15])

                # ---- phase C: T-layout rhs packs (batched over jt) ---
                hmrhs = sml.tile([128, 3 * 304], bf16, tag="hmrhs")
                ptsa = sml.tile([128, 600], bf16, tag="ptsa")
                p4ua = sml.tile([128, 600], bf16, tag="p4ua")
                gsrc = sml.tile([128, 312], bf16, tag="gsrc")
                for jt in range(NT):
                    ctp = ps.tile([128, 384], f32, tag="ctt")
                    nc.tensor.matmul(ctp[:],
                                     qsb16[:, jt * 128 : (jt + 1) * 128],
                                     psb16[:], start=True, stop=True)
                    nc.scalar.activation(cts16[:, jt * S : (jt + 1) * S],
                                         ctp[:], AF.Copy)
                tqp = ps.tile([128, 672], bf16, tag="misc")
                for jt in range(NT):
                    nc.tensor.transpose(tqp[:, jt * 224 : jt * 224 + H],
                                        qsb16[:, jt * 128 : (jt + 1) * 128],
                                        idn16[:H, :H])
                    nc.tensor.transpose(
                        tqp[:, jt * 224 + 112 : jt * 224 + 112 + H],
                        psb16[:, jt * 128 : (jt + 1) * 128],
                        idn16[:H, :H])
                nc.vector.tensor_tensor(
                    ap3(hmrhs, 0, [[304, 3], [1, H]]),
                    ap3(tqp, 0, [[224, 3], [1, H]]),
                    ap3(iq4t, 0, [[3, 3], [0, H]]), op=ALU.mult)
                # q3uT_l = qT * u3_l for l=0,1 (read qT twice per jt)
                nc.vector.tensor_tensor(
                    ap3(hmrhs, H, [[304, 3], [1, 2 * H]]),
                    ap3(hmrhs, 0, [[304, 3], [0, 2], [1, H]]),
                    ap3(ubc16, 0, [[0, 3], [1, 2 * H]]), op=ALU.mult)
                nc.scalar.activation(ap3(gsrc, 0, [[104, 3], [1, H]]),
                                     ap3(tqp, 0, [[224, 3], [1, H]]),
                                     AF.Copy)
                nc.vector.tensor_copy(ap3(gsrc, H, [[104, 3], [1, 2]]),
                                      ap3(iq4t, 1, [[3, 3], [1, 2]]))
                nc.vector.memset(ap3(gsrc, H + 2, [[104, 3], [1, 2]]), 0.0)
                nc.sync.dma_start(
                    AP(GT, 0, [[104, 128], [128 * 104, 3], [1, 104]]),
                    ap3(gsrc, 0, [[104, 3], [1, 104]]))
                nc.scalar.activation(ap3(ptsa, 0, [[200, 3], [1, H]]),
                                     ap3(tqp, 112, [[224, 3], [1, H]]),
                                     AF.Copy)
                # p4uT pack = [pT*u4_0 | pT*u4_1] per it
                nc.vector.tensor_tensor(
                    p4ua[:],
                    ap3(ptsa, 0, [[200, 3], [0, 2], [1, H]]),
                    ap3(ubc16, 2 * H, [[0, 3], [1, 2 * H]]), op=ALU.mult)

                # ---- phase D: hm matmuls, gather, dot products -------
                # m3 = sign(sum_att) * n3_raw / sqrt(d3_raw): the 1/sum
                # magnitude cancels in the cosine, only its sign matters
                # (applied in phase E), so no reciprocal is needed here.
                pscr = sml.tile([128, 1200], f32, tag="pscr")
                gall = sml.tile([128, 312], bf16, tag="gout")
                hmxa = psm.tile([128, 1536], f32, tag="mega")
                for it in range(NT):
                    for jt in range(NT):
                        nc.tensor.matmul(
                            hmxa[:, it * 512 : it * 512 + 300],
                            cts16[:, jt * S + it * 128
                                  : jt * S + (it + 1) * 128],
                            hmrhs[:, jt * 304 : jt * 304 + 300],
                            start=(jt == 0), stop=(jt == NT - 1))
                    nc.gpsimd.indirect_dma_start(
                        out=gall[:, it * 104 : (it + 1) * 104],
                        out_offset=None, in_=GT.ap(),
                        in_offset=bass.IndirectOffsetOnAxis(
                            ap=idx8[it][:, 0:1], axis=0))
                # hm (raw) -> next to pT for the m3 d3 products
                nc.scalar.activation(ap3(ptsa, H, [[200, 3], [1, H]]),
                                     ap3(hmxa, 0, [[512, 3], [1, H]]),
                                     AF.Copy)
                # m3 numerator/denominator products, all its at once
                nc.vector.tensor_tensor(
                    pscr[:, 0:600],
                    ap3(hmxa, H, [[512, 3], [1, 2 * H]]),
                    ap3(ptsa, 0, [[200, 3], [0, 2], [1, H]]), op=ALU.mult)
                nc.vector.tensor_tensor(
                    pscr[:, 600:1200],
                    ap3(hmxa, H, [[512, 3], [1, 2 * H]]),
                    ap3(ptsa, H, [[200, 3], [0, 2], [1, H]]), op=ALU.mult)
                nc.vector.reduce_sum(
                    fscap(0, [[16, 3], [1, 2]]),
                    ap3(pscr, 0, [[100, 6], [1, H]]), axis=AX.X)
                nc.vector.reduce_sum(
                    fscap(4, [[16, 3], [1, 2]]),
                    ap3(pscr, 600, [[100, 6], [1, H]]), axis=AX.X)
                # m4 products: hmaxT (read twice) * [p4uT_0 | p4uT_1]
                nc.vector.tensor_tensor(
                    pscr[:, 0:600],
                    ap3(gall, 0, [[104, 3], [0, 2], [1, H]]),
                    p4ua[:], op=ALU.mult)
                nc.vector.reduce_sum(
                    fscap(2, [[16, 3], [1, 2]]),
                    ap3(pscr, 0, [[200, 3], [100, 2], [1, H]]), axis=AX.X)
                nc.vector.tensor_copy(
                    fscap(6, [[16, 3], [1, 2]]),
                    ap3(gall, H, [[104, 3], [1, 2]]))

            # ---- phase E: finalize this direction's 8 pairs ----------
            if sub != NSUB - 1:
                continue
            fof = di * BC * 48
            nc.scalar.activation(
                ap3(fscall, fof + 13, [[48, BC], [16, 3], [1, 1]]),
                ap3(fscall, fof + 14, [[48, BC], [16, 3], [1, 1]]), AF.Sign)
            den = ap3(fscall, fof + 4, [[48, BC], [16, 3], [1, 2]])
            nc.scalar.activation(den, den, AF.Sqrt)
            nc.vector.reciprocal(den, den)
            # m3/m4: tmp34 = num * invden, then * transposed 1/|w p| norms
            nc.vector.tensor_tensor(
                ap3(tmp34, di * BC * 12, [[12, BC], [1, 12]]),
                ap3(fscall, fof, [[48, BC], [16, 3], [1, 4]]),
                ap3(fscall, fof + 4, [[48, BC], [16, 3], [1, 4]]),
                op=ALU.mult)
            nc.vector.tensor_tensor(
                ap3(tmp34, di * BC * 12, [[12, BC], [4, 3], [1, 2]]),
                ap3(tmp34, di * BC * 12, [[12, BC], [4, 3], [1, 2]]),
                ap3(fscall, fof + 13, [[48, BC], [16, 3], [0, 2]]),
                op=ALU.mult)
            for o in range(2):  # o=0: m3, o=1: m4
                nc.vector.tensor_tensor(
                    ap3(stgt, (4 + di) * 48 + 96 * o,
                        [[2, BC], [16, 3], [1, 2]]),
                    ap3(tmp34, di * BC * 12 + o * 2,
                        [[12, BC], [4, 3], [1, 2]]),
                    ap3(ipnall, di * BC * 27 + (7 if o == 0 else 1),
                        [[27, BC], [9, 3], [1, 2]]),
                    op=ALU.mult)
            # m1/m2: tmp12 = num * 1/|w p|, then m1 *= 1/nq
            for o in range(2):  # o=0: m1, o=1: m2
                nc.vector.tensor_tensor(
                    ap3(tmp12, di * BC * 12 + o * 2,
                        [[12, BC], [4, 3], [1, 2]]),
                    ap3(fscall, fof + (11 if o == 0 else 8),
                        [[48, BC], [16, 3], [1, 2]]),
                    ap3(ipnall, di * BC * 27 + (5 if o == 0 else 3),
                        [[27, BC], [9, 3], [1, 2]]),
                    op=ALU.mult)
                nc.vector.tensor_tensor(
                    ap3(stgt, di * 48 + 96 * o, [[2, BC], [16, 3], [1, 2]]),
                    ap3(tmp12, di * BC * 12 + o * 2,
                        [[12, BC], [4, 3], [1, 2]]),
                    ap3(nq4all, di * BC * 4 + (0 if o == 0 else 2),
                        [[4, BC], [0, 3], [1, 2]]),
                    op=ALU.mult)

            for m, nm in enumerate(
                    (("m1f", "m2f", "m3f", "m4f"),
                     ("m1b", "m2b", "m3b", "m4b"))[di]):
                oi = 2 * m + di
                t = outs[nm]
                out_ap = AP(t, 0, [[16, 128], [2048, 3], [1, 16]])
                in_ap = ap3(stgt, oi * 48, [[16, 3], [1, 16]])
                nc.sync.dma_start(out_ap, in_ap)

        for p in (psm, ps, nrm, pq, stg, sml, big, con):
            p.release()

    nc.compile()
    return nc


def _host_consts(w1, w2, w3f_, w4f_):
    u1 = (w1 * w1).astype(np.float32)
    u2 = (w2 * w2).astype(np.float32)
    u3 = (w3f_ * w3f_).astype(np.float32)
    u4 = (w4f_ * w4f_).astype(np.float32)
    ones = np.ones((H,), np.float32)
    wsb = np.stack([ones, u4[0], u4[1], u2[0], u2[1],
                    u1[0], u1[1], u3[0], u3[1]], axis=1)  # (H, 9)
    return u1, u2, u3, u4, wsb


def _prepare(p_f, p_b, q_f, q_b, w1f, w1b, w2f, w2b, w3f, w3b, w4f, w4b):
    import ml_dtypes

    bfd = ml_dtypes.bfloat16
    if "prog" not in _COMPILED:
        _COMPILED["prog"] = _build_program()
    nc = _COMPILED["prog"]

    p_f, p_b = np.asarray(p_f), np.asarray(p_b)
    q_f, q_b = np.asarray(q_f), np.asarray(q_b)
    u1f, u2f_, u3, u4, wsbf = _host_consts(
        np.asarray(w1f), np.asarray(w2f), np.asarray(w3f), np.asarray(w4f))
    u1b, u2b_, _, _, wsbb = _host_consts(
        np.asarray(w1b), np.asarray(w2b), np.asarray(w3f), np.asarray(w4f))
    WSB = np.concatenate([wsbf, wsbb], axis=1).astype(np.float32)
    U1C = np.stack([u1f[0], u1f[1], u1b[0], u1b[1]], 1).astype(np.float32)
    U2C = np.stack([u2f_[0], u2f_[1], u2b_[0], u2b_[1]], 1).astype(np.float32)
    ubc = np.concatenate([u3[0], u3[1], u4[0], u4[1]]).astype(np.float32)
    UBC16 = np.ascontiguousarray(
        np.broadcast_to(ubc, (128, 400))).astype(bfd)
    SEL16 = np.zeros((9, 384), np.float32)
    SEL16[3, 128:256] = 1.0  # invq2_0
    SEL16[4, 256:384] = 1.0  # invq2_1
    IDN = np.eye(128, dtype=np.float32)

    in_maps = []
    for c in range(NCORES):
        sl = slice(c * BC, (c + 1) * BC)
        in_maps.append({
            "p_f": np.ascontiguousarray(p_f[:, sl, :]),
            "p_b": np.ascontiguousarray(p_b[:, sl, :]),
            "q_f": np.ascontiguousarray(q_f[:, sl, :]),
            "q_b": np.ascontiguousarray(q_b[:, sl, :]),
            "WSB16": WSB.astype(bfd), "WSBF": WSB,
            "U1C": U1C, "U2C": U2C, "UBC16": UBC16,
            "SEL16": SEL16.astype(bfd),
            "IDN": IDN, "IDN16": IDN.astype(bfd),
        })

    return nc, in_maps


def _gather_outputs(results):
    full = []
    for nm in ("m1f", "m1b", "m2f", "m2b", "m3f", "m3b", "m4f", "m4b"):
        full.append(np.concatenate([results[c][nm] for c in range(NCORES)],
                                   axis=1))
    return tuple(full)


def kernel(**inputs):
    from concourse.bass_utils import run_bass_kernel_spmd

    nc, in_maps = _prepare(**inputs)
    res = run_bass_kernel_spmd(nc, in_maps, list(range(NCORES)))
    return _gather_outputs(res.results)


def run_traced(**inputs):
    """Run with NTFF profiling; returns (exec_time_ns, results_obj)."""
    from concourse.bass_utils import run_bass_kernel_spmd

    nc, in_maps = _prepare(**inputs)
    res = run_bass_kernel_spmd(nc, in_maps, list(range(NCORES)), trace=True)
    return res.exec_time_ns, res


# revision 53
# speedup vs baseline: 1.6715x; 1.0019x over previous
"""BiMPM matching-layer kernel for 8 Trainium2 NeuronCores.

Data-parallel over the batch axis: each of the 8 cores gets 8 of the 64
batch elements (full hidden=100 and seq=384 on every core). Weights are
replicated (host-squared into the forms the device needs).

Per (batch, direction): m1 (cosine vs last q timestep), m2 (max pairwise
weighted cosine), m3 (cosine vs attention-mean of q), m4 (cosine vs
argmax-attended q), for l in {0,1}. The backward direction reuses the
*forward* w3/w4 tables (reference bug preserved). Outputs: 8 tensors of
shape (384, 64, 2), order m1f,m1b,m2f,m2b,m3f,m3b,m4f,m4b.

Precision plan: the m3 sign (1/sum of attention) and the m4 argmax are
ill-conditioned (gaps ~1e-4 / ~1e-6), so the p-q̂ attention matmul and
the q norms run in full fp32. Everything else (m2 numerators, hmean,
m4 dot products, p norms) tolerates bf16, which runs the PE at 1
cycle/row with fast weight loads instead of fp32's 4 cycles/row.
"""

import os
import sys

sys.path.insert(0, "/opt/trn_rl_repo")

import numpy as np

H, B, S, L = 100, 64, 384, 2
NCORES = 8
BC = B // NCORES  # 8 batches per core
NT = S // 128  # 3 tiles of 128 along seq
NPAIR = 2 * BC  # 16 (direction, batch) pairs per core

_COMPILED = {}


def _build_program():
    """Builds the single-core SPMD Bass program (same on all 8 cores)."""
    import concourse.bacc as bacc
    import concourse.bass as bass
    import concourse.mybir as mybir
    import concourse.tile as tile
    from concourse.bass_types import AP

    dt = mybir.dt
    f32 = dt.float32
    bf16 = dt.bfloat16
    AF = mybir.ActivationFunctionType
    ALU = mybir.AluOpType
    AX = mybir.AxisListType

    nc = bacc.Bacc("TRN2", target_bir_lowering=False, debug=False)

    ins = {}
    for nm in ("p_f", "p_b", "q_f", "q_b"):
        ins[nm] = nc.dram_tensor(nm, [H, BC, S], f32, kind="ExternalInput")
    WSB16 = nc.dram_tensor("WSB16", [H, 18], bf16, kind="ExternalInput")
    WSBF = nc.dram_tensor("WSBF", [H, 18], f32, kind="ExternalInput")
    U1C = nc.dram_tensor("U1C", [H, 4], f32, kind="ExternalInput")
    U2C = nc.dram_tensor("U2C", [H, 4], f32, kind="ExternalInput")
    UBC16 = nc.dram_tensor("UBC16", [128, 400], bf16, kind="ExternalInput")
    SEL16 = nc.dram_tensor("SEL16", [9, 384], bf16, kind="ExternalInput")
    IDN = nc.dram_tensor("IDN", [128, 128], f32, kind="ExternalInput")
    IDN16 = nc.dram_tensor("IDN16", [128, 128], bf16, kind="ExternalInput")
    outs = {}
    for nm in ("m1f", "m1b", "m2f", "m2b", "m3f", "m3b", "m4f", "m4b"):
        outs[nm] = nc.dram_tensor(nm, [S, BC, L], f32, kind="ExternalOutput")
    # per-pair DRAM scratch for the gather source (bf16 rows)
    GTs = [nc.dram_tensor(f"GT{i}", [S, 104], bf16) for i in range(NPAIR)]

    def ap3(t, off, pattern):
        """AP on tile t: partition dim + explicit free-dim [step,count]s."""
        base = t[:, 0:1]
        part = list(base.ap[0])
        return AP(base.tensor, base.offset + off,
                  [part] + [list(x) for x in pattern])

    with tile.TileContext(nc) as tc:
        con = tc.alloc_tile_pool(name="con", bufs=1)
        big = tc.alloc_tile_pool(name="big", bufs=3)
        sml = tc.alloc_tile_pool(name="sml", bufs=3)
        stg = tc.alloc_tile_pool(name="stg", bufs=1)
        ps = tc.alloc_tile_pool(name="ps", bufs=1, space="PSUM")
        psm = tc.alloc_tile_pool(name="psm", bufs=2, space="PSUM")

        # ---- constants -----------------------------------------------
        idn = con.tile([128, 128], f32, tag="idn")
        nc.sync.dma_start(idn[:], IDN.ap())
        idn16 = con.tile([128, 128], bf16, tag="idn16")
        nc.sync.dma_start(idn16[:], IDN16.ap())
        onesb = con.tile([128, 128], f32, tag="onesb")
        nc.vector.memset(onesb[0:1, :], 1.0)
        ones_row = onesb[0:1, :]
        wsb16_t = con.tile([128, 18], bf16, tag="wsb16")
        nc.sync.dma_start(wsb16_t[0:H, :], WSB16.ap())
        wsb16 = wsb16_t[0:H, :]
        wsbff_t = con.tile([128, 18], f32, tag="wsbff")
        nc.sync.dma_start(wsbff_t[0:H, :], WSBF.ap())
        wsbff = wsbff_t[0:H, :]
        u1c_t = con.tile([128, 4], f32, tag="u1c")
        nc.sync.dma_start(u1c_t[0:H, :], U1C.ap())
        u1c = u1c_t[0:H, :]
        u2c_t = con.tile([128, 4], f32, tag="u2c")
        nc.sync.dma_start(u2c_t[0:H, :], U2C.ap())
        u2c = u2c_t[0:H, :]
        ubc16 = con.tile([128, 400], bf16, tag="ubc16")
        nc.sync.dma_start(ubc16[:], UBC16.ap())
        sel_t = con.tile([128, 384], bf16, tag="sel16")
        nc.sync.dma_start(sel_t[0:9, :], SEL16.ap())
        sel16 = sel_t[0:9, :]
        zeros8 = con.tile([128, 8], f32, tag="zeros8")
        nc.vector.memset(zeros8[:], 0.0)

        # ---- persistent staging --------------------------------------
        # output staging: col = out_idx*48 + it*16 + b*2 + l
        stgt = stg.tile([128, 384], f32, tag="stgt")
        # per-pair scalars: col block pr*48 (layout matches old fsc)
        fscall = stg.tile([128, NPAIR * 48], f32, tag="fscall")
        # per-pair transposed inverse norms (27 cols each)
        ipnall = stg.tile([128, NPAIR * 27], f32, tag="ipnall")
        # per-pair [1/nq_0, 1/nq_1, 1, 1]
        nq4all = stg.tile([128, NPAIR * 4], f32, tag="nq4all")
        nc.vector.memset(ap3(nq4all, 2, [[4, NPAIR], [1, 2]]), 1.0)
        tmp34 = stg.tile([128, NPAIR * 12], f32, tag="tmp34")
        tmp12 = stg.tile([128, NPAIR * 12], f32, tag="tmp12")
        # p/q fp32 loads persist across pass 1 and pass 2
        pq = tc.alloc_tile_pool(name="pq", bufs=BC + 1)
        # batched-norm workspace (4 pairs at a time, double-buffered)
        nrm = tc.alloc_tile_pool(name="nrm", bufs=2)
        NSUB = 2
        QB = BC // NSUB

        from concourse.dve_ops import RECIPROCAL_APPROX_NR

        for di, d in enumerate(("f", "b")):
          P_in, Q_in = ins["p_" + d], ins["q_" + d]
          for sub in range(NSUB):
            b0 = sub * QB
            pnall = nrm.tile([128, QB * 768], f32, tag="pnall")
            ipqa = nrm.tile([128, QB * 768], f32, tag="ipqa")
            rsca = nrm.tile([128, QB * 768], f32, tag="rsca")
            iqr16a = nrm.tile([128, QB * 384], bf16, tag="iqr16a")
            if True:
            # ---- pass 1: loads, squares, norm matmuls ----------------
              pqs = []
              for b in range(b0, b0 + QB):
                pr = di * BC + b
                psbf_t = pq.tile([128, S], f32, tag="psbf")
                qsbf_t = pq.tile([128, S], f32, tag="qsbf")
                psbf, qsbf = psbf_t[0:H, :], qsbf_t[0:H, :]
                pqs.append((psbf, qsbf))
                nc.sync.dma_start(psbf[:], P_in.ap()[:, b, :])
                nc.sync.dma_start(qsbf[:], Q_in.ap()[:, b, :])
                psq16_t = big.tile([128, S], bf16, tag="psq16")
                psq16 = psq16_t[0:H, :]
                nc.scalar.activation(psq16[:], psbf[:], AF.Square)
                qsq_t = big.tile([128, S], f32, tag="qsq")
                qsq = qsq_t[0:H, :]
                nc.gpsimd.tensor_tensor(qsq[:], qsbf[:], qsbf[:],
                                        op=ALU.mult)
                pnqn = psm.tile([9, 1536], f32, tag="mega")
                nc.tensor.matmul(pnqn[:, 0:S],
                                 wsb16[:, di * 9 : di * 9 + 9], psq16[:],
                                 start=True, stop=True)
                nc.tensor.matmul(pnqn[:, 512 : 512 + S],
                                 wsbff[:, di * 9 : di * 9 + 9], qsq[:],
                                 start=True, stop=True)
                nc.scalar.activation(
                    pnall[0:9, (b - b0) * 768 : (b - b0 + 1) * 768],
                    ap3(pnqn, 0, [[512, 2], [1, S]]), AF.Copy)
                # m1 q-side norm: nq^2 = sum(u1_l * qlast^2)
                misc = ps.tile([128, 512], f32, tag="misc")
                nc.tensor.matmul(misc[0:1, 32:34], qsq[:, S - 1 : S],
                                 u1c[:, di * 2 : di * 2 + 2],
                                 start=True, stop=True)
                nqr = sml.tile([128, 2], f32, tag="nqr")
                nc.scalar.activation(nqr[0:1, :], misc[0:1, 32:34], AF.Sqrt)
                nc.vector.reciprocal(nqr[0:1, :], nqr[0:1, :])
                nc.tensor.matmul(misc[:, 40:42], ones_row[:],
                                 nqr[0:1, :], start=True, stop=True)
                nc.vector.tensor_copy(nq4all[:, pr * 4 : pr * 4 + 2],
                                      misc[:, 40:42])

            # ---- batched 1/sqrt pipeline over the 4 pairs ------------
            pn9, ip9, rs9 = pnall[0:9, :], ipqa[0:9, :], rsca[0:9, :]
            nc.scalar.activation(rs9[:], pn9[:], AF.Sqrt)
            nc.vector.reciprocal_approx_fast(out=ip9[:], in_=rs9[:])
            # only iqn row 0 (1/|q|, argmax-critical) needs more than the
            # ~3e-4 LUT+approx accuracy: NR the reciprocal, then one
            # rsqrt Newton polish against the exact norm^2.
            ip0 = ap3(ipqa[0:1, :], S, [[768, QB], [1, S]])
            rs0 = ap3(rsca[0:1, :], S, [[768, QB], [1, S]])
            nc.vector._custom_dve(RECIPROCAL_APPROX_NR, out=ip0,
                                  in0=rs0, in1=ip0, s0=2.0)
            nc.vector._custom_dve(RECIPROCAL_APPROX_NR, out=ip0,
                                  in0=rs0, in1=ip0, s0=2.0)
            nraap = ap3(rsca[0:1, :], S, [[768, QB], [1, S]])
            pnq = ap3(pnall[0:1, :], S, [[768, QB], [1, S]])
            nc.vector.tensor_tensor(nraap, pnq, ip0, op=ALU.mult)
            nc.vector.tensor_tensor(nraap, nraap, ip0, op=ALU.mult)
            nc.vector.tensor_scalar(nraap, nraap, -0.5, 1.5,
                                    op0=ALU.mult, op1=ALU.add)
            nc.vector.tensor_tensor(ip0, ip0, nraap, op=ALU.mult)
            nc.vector.tensor_copy(iqr16a[0:5, :],
                                  ap3(ipqa[0:5, :], S, [[768, QB], [1, S]]))

            # ---- pass 2: per-pair compute ----------------------------
            if True:
              for b in range(b0, b0 + QB):
                pr = di * BC + b
                bl = b - b0
                GT = GTs[pr]

                def fscap(off, pattern):
                    return ap3(fscall, pr * 48 + off, pattern)

                fsc = fscall[:, pr * 48 : pr * 48 + 48]
                ipn = ipqa[0:9, bl * 768 : bl * 768 + S]
                iqn = ipqa[0:9, bl * 768 + S : bl * 768 + 2 * S]
                iqr16 = iqr16a[0:5, bl * 384 : (bl + 1) * 384]

                psbf, qsbf = pqs[bl]
                psb16_t = big.tile([128, S], bf16, tag="psb16")
                qsb16_t = big.tile([128, S], bf16, tag="qsb16")
                psb16, qsb16 = psb16_t[0:H, :], qsb16_t[0:H, :]
                nc.scalar.activation(psb16[:], psbf[:], AF.Copy)
                nc.scalar.activation(qsb16[:], qsbf[:], AF.Copy)

                # transposed inverse norms
                misc = ps.tile([128, 512], f32, tag="misc")
                for it in range(NT):
                    nc.tensor.transpose(misc[:, it * 9 : it * 9 + 9],
                                        ipn[:, it * 128 : (it + 1) * 128],
                                        idn[:9, :9])
                for it in range(NT):
                    nc.tensor.transpose(
                        misc[:, 64 + it * 3 : 64 + it * 3 + 3],
                        iqn[0:3, it * 128 : (it + 1) * 128],
                        idn[:3, :3])
                nc.scalar.activation(ipnall[:, pr * 27 : pr * 27 + 27],
                                     misc[:, 0:27], AF.Copy)
                iq4t = sml.tile([128, 9], f32, tag="iq4t")
                nc.scalar.activation(iq4t[:], misc[:, 64:73], AF.Copy)

                # q variants: qn = q/|q| (fp32), q2n = q*u2_l/|w2_l q| (bf16)
                qn_t = big.tile([128, S + 4], f32, tag="qn")
                qn = qn_t[0:H, :]
                q2a_t = big.tile([128, 2 * S], bf16, tag="q2a")
                q2n_t = big.tile([128, 2 * S], bf16, tag="q2n")
                q2a, q2n = q2a_t[0:H, :], q2n_t[0:H, :]
                nc.scalar.activation(q2a[:, 0:S], qsbf[:], AF.Copy,
                                     scale=u2c[:, di * 2 : di * 2 + 1])
                nc.scalar.activation(q2a[:, S : 2 * S], qsbf[:], AF.Copy,
                                     scale=u2c[:, di * 2 + 1 : di * 2 + 2])
                bcq = sml.tile([128, S], f32, tag="bcq")
                nc.gpsimd.partition_broadcast(bcq[0:H, :], iqn[0:1, :])
                nc.vector.tensor_tensor(qn[:, 0:S], qsbf[:], bcq[0:H, :],
                                        op=ALU.mult)
                nc.vector.tensor_tensor(
                    qn[:, S : S + 2],
                    qsbf[:, S - 1 : S].to_broadcast([H, 2]),
                    u1c[:, di * 2 : di * 2 + 2],
                    op=ALU.mult)

                for k in (1, 2):
                    bc = ps.tile([128, 384], f32, tag="ctt")
                    nc.tensor.matmul(
                        bc[0:H, 0:S],
                        sel16[0:5, k * 128 : k * 128 + H],
                        iqr16[:], start=True, stop=True)
                    off = (k - 1) * S
                    nc.vector.tensor_tensor(q2n[:, off : off + S],
                                            q2a[:, off : off + S],
                                            bc[0:H, 0:S], op=ALU.mult)

                # ---- phase B: i-layout matmuls + fused maxes ---------
                idx8 = [sml.tile([128, 8], dt.uint32, tag=f"idx{it}",
                                 name=f"idx{it}")
                        for it in range(NT)]
                cts16 = big.tile([128, NT * S], bf16, tag="cts16")
                for it in range(NT):
                    mega = psm.tile([128, 1536], f32, tag="mega")
                    lhs16 = psb16[:, it * 128 : (it + 1) * 128]
                    nc.tensor.matmul(mega[:, 0:S], lhs16,
                                     q2n[:, 0:S], start=True, stop=True)
                    nc.tensor.matmul(mega[:, 512 : 512 + S], lhs16,
                                     q2n[:, S : 2 * S],
                                     start=True, stop=True)
                    nc.tensor.matmul(
                        mega[:, 1024 : 1024 + S + 2],
                        psbf[:, it * 128 : (it + 1) * 128],
                        qn[:, 0 : S + 2], start=True, stop=True)
                    nc.vector.reduce_max(
                        fsc[:, it * 16 + 8 : it * 16 + 11],
                        ap3(mega, 0, [[512, 3], [1, S]]), axis=AX.X)
                    nc.vector.max_index(
                        idx8[it][:], fscap(it * 16 + 10, [[0, 8]]),
                        mega[:, 1024 : 1024 + S])
                    nc.scalar.activation(
                        fsc[:, it * 16 + 11 : it * 16 + 13],
                        mega[:, 1024 + S : 1024 + S + 2], AF.Copy)
                    att16 = big.tile([128, S], bf16, tag="att16")
                    nc.scalar.activation(
                        att16[:], mega[:, 1024 : 1024 + S], AF.Copy,
                        accum_out=fsc[:, it * 16 + 14 : it * 16 + 15])

                # ---- phase C: T-layout rhs packs (batched over jt) ---
                hmrhs = sml.tile([128, 3 * 304], bf16, tag="hmrhs")
                ptsa = sml.tile([128, 600], bf16, tag="ptsa")
                p4ua = sml.tile([128, 600], bf16, tag="p4ua")
                gsrc = sml.tile([128, 312], bf16, tag="gsrc")
                for jt in range(NT):
                    ctp = ps.tile([128, 384], f32, tag="ctt")
                    nc.tensor.matmul(ctp[:],
                                     qsb16[:, jt * 128 : (jt + 1) * 128],
                                     psb16[:], start=True, stop=True)
                    nc.scalar.activation(cts16[:, jt * S : (jt + 1) * S],
                                         ctp[:], AF.Copy)
                tqp = ps.tile([128, 672], bf16, tag="misc")
                for jt in range(NT):
                    nc.tensor.transpose(tqp[:, jt * 224 : jt * 224 + H],
                                        qsb16[:, jt * 128 : (jt + 1) * 128],
                                        idn16[:H, :H])
                    nc.tensor.transpose(
                        tqp[:, jt * 224 + 112 : jt * 224 + 112 + H],
                        psb16[:, jt * 128 : (jt + 1) * 128],
                        idn16[:H, :H])
                nc.vector.tensor_tensor(
                    ap3(hmrhs, 0, [[304, 3], [1, H]]),
                    ap3(tqp, 0, [[224, 3], [1, H]]),
                    ap3(iq4t, 0, [[3, 3], [0, H]]), op=ALU.mult)
                # q3uT_l = qT * u3_l for l=0,1 (read qT twice per jt)
                nc.vector.tensor_tensor(
                    ap3(hmrhs, H, [[304, 3], [1, 2 * H]]),
                    ap3(hmrhs, 0, [[304, 3], [0, 2], [1, H]]),
                    ap3(ubc16, 0, [[0, 3], [1, 2 * H]]), op=ALU.mult)
                nc.scalar.activation(ap3(gsrc, 0, [[104, 3], [1, H]]),
                                     ap3(tqp, 0, [[224, 3], [1, H]]),
                                     AF.Copy)
                nc.vector.tensor_copy(ap3(gsrc, H, [[104, 3], [1, 2]]),
                                      ap3(iq4t, 1, [[3, 3], [1, 2]]))
                nc.vector.memset(ap3(gsrc, H + 2, [[104, 3], [1, 2]]), 0.0)
                nc.sync.dma_start(
                    AP(GT, 0, [[104, 128], [128 * 104, 3], [1, 104]]),
                    ap3(gsrc, 0, [[104, 3], [1, 104]]))
                nc.scalar.activation(ap3(ptsa, 0, [[200, 3], [1, H]]),
                                     ap3(tqp, 112, [[224, 3], [1, H]]),
                                     AF.Copy)
                # p4uT pack = [pT*u4_0 | pT*u4_1] per it
                nc.vector.tensor_tensor(
                    p4ua[:],
                    ap3(ptsa, 0, [[200, 3], [0, 2], [1, H]]),
                    ap3(ubc16, 2 * H, [[0, 3], [1, 2 * H]]), op=ALU.mult)

                # ---- phase D: hm matmuls, gather, dot products -------
                # m3 = sign(sum_att) * n3_raw / sqrt(d3_raw): the 1/sum
                # magnitude cancels in the cosine, only its sign matters
                # (applied in phase E), so no reciprocal is needed here.
                pscr = sml.tile([128, 1200], f32, tag="pscr")
                gall = sml.tile([128, 312], bf16, tag="gout")
                hmxa = psm.tile([128, 1536], f32, tag="mega")
                for it in range(NT):
                    for jt in range(NT):
                        nc.tensor.matmul(
                            hmxa[:, it * 512 : it * 512 + 300],
                            cts16[:, jt * S + it * 128
                                  : jt * S + (it + 1) * 128],
                            hmrhs[:, jt * 304 : jt * 304 + 300],
                            start=(jt == 0), stop=(jt == NT - 1))
                    nc.gpsimd.indirect_dma_start(
                        out=gall[:, it * 104 : (it + 1) * 104],
                        out_offset=None, in_=GT.ap(),
                        in_offset=bass.IndirectOffsetOnAxis(
                            ap=idx8[it][:, 0:1], axis=0))
                # hm (raw) -> next to pT for the m3 d3 products
                nc.scalar.activation(ap3(ptsa, H, [[200, 3], [1, H]]),
                                     ap3(hmxa, 0, [[512, 3], [1, H]]),
                                     AF.Copy)
                # m3 numerator/denominator products, all its at once
                nc.vector.tensor_tensor(
                    pscr[:, 0:600],
                    ap3(hmxa, H, [[512, 3], [1, 2 * H]]),
                    ap3(ptsa, 0, [[200, 3], [0, 2], [1, H]]), op=ALU.mult)
                nc.vector.tensor_tensor(
                    pscr[:, 600:1200],
                    ap3(hmxa, H, [[512, 3], [1, 2 * H]]),
                    ap3(ptsa, H, [[200, 3], [0, 2], [1, H]]), op=ALU.mult)
                nc.vector.reduce_sum(
                    fscap(0, [[16, 3], [1, 2]]),
                    ap3(pscr, 0, [[100, 6], [1, H]]), axis=AX.X)
                nc.vector.reduce_sum(
                    fscap(4, [[16, 3], [1, 2]]),
                    ap3(pscr, 600, [[100, 6], [1, H]]), axis=AX.X)
                # m4 products: hmaxT (read twice) * [p4uT_0 | p4uT_1]
                nc.vector.tensor_tensor(
                    pscr[:, 0:600],
                    ap3(gall, 0, [[104, 3], [0, 2], [1, H]]),
                    p4ua[:], op=ALU.mult)
                nc.vector.reduce_sum(
                    fscap(2, [[16, 3], [1, 2]]),
                    ap3(pscr, 0, [[200, 3], [100, 2], [1, H]]), axis=AX.X)
                nc.vector.tensor_copy(
                    fscap(6, [[16, 3], [1, 2]]),
                    ap3(gall, H, [[104, 3], [1, 2]]))

            # ---- phase E: finalize this direction's 8 pairs ----------
            if sub != NSUB - 1:
                continue
            fof = di * BC * 48
            nc.scalar.activation(
                ap3(fscall, fof + 13, [[48, BC], [16, 3], [1, 1]]),
                ap3(fscall, fof + 14, [[48, BC], [16, 3], [1, 1]]), AF.Sign)
            den = ap3(fscall, fof + 4, [[48, BC], [16, 3], [1, 2]])
            nc.scalar.activation(den, den, AF.Sqrt)
            nc.vector.reciprocal(den, den)
            # m3/m4: tmp34 = num * invden, then * transposed 1/|w p| norms
            nc.vector.tensor_tensor(
                ap3(tmp34, di * BC * 12, [[12, BC], [1, 12]]),
                ap3(fscall, fof, [[48, BC], [16, 3], [1, 4]]),
                ap3(fscall, fof + 4, [[48, BC], [16, 3], [1, 4]]),
                op=ALU.mult)
            nc.vector.tensor_tensor(
                ap3(tmp34, di * BC * 12, [[12, BC], [4, 3], [1, 2]]),
                ap3(tmp34, di * BC * 12, [[12, BC], [4, 3], [1, 2]]),
                ap3(fscall, fof + 13, [[48, BC], [16, 3], [0, 2]]),
                op=ALU.mult)
            for o in range(2):  # o=0: m3, o=1: m4
                nc.vector.tensor_tensor(
                    ap3(stgt, (4 + di) * 48 + 96 * o,
                        [[2, BC], [16, 3], [1, 2]]),
                    ap3(tmp34, di * BC * 12 + o * 2,
                        [[12, BC], [4, 3], [1, 2]]),
                    ap3(ipnall, di * BC * 27 + (7 if o == 0 else 1),
                        [[27, BC], [9, 3], [1, 2]]),
                    op=ALU.mult)
            # m1/m2: tmp12 = num * 1/|w p|, then m1 *= 1/nq
            for o in range(2):  # o=0: m1, o=1: m2
                nc.vector.tensor_tensor(
                    ap3(tmp12, di * BC * 12 + o * 2,
                        [[12, BC], [4, 3], [1, 2]]),
                    ap3(fscall, fof + (11 if o == 0 else 8),
                        [[48, BC], [16, 3], [1, 2]]),
                    ap3(ipnall, di * BC * 27 + (5 if o == 0 else 3),
                        [[27, BC], [9, 3], [1, 2]]),
                    op=ALU.mult)
                nc.vector.tensor_tensor(
                    ap3(stgt, di * 48 + 96 * o, [[2, BC], [16, 3], [1, 2]]),
                    ap3(tmp12, di * BC * 12 + o * 2,
                        [[12, BC], [4, 3], [1, 2]]),
                    ap3(nq4all, di * BC * 4 + (0 if o == 0 else 2),
                        [[4, BC], [0, 3], [1, 2]]),
                    op=ALU.mult)

            for m, nm in enumerate(
                    (("m1f", "m2f", "m3f", "m4f"),
                     ("m1b", "m2b", "m3b", "m4b"))[di]):
                oi = 2 * m + di
                t = outs[nm]
                out_ap = AP(t, 0, [[16, 128], [2048, 3], [1, 16]])
                in_ap = ap3(stgt, oi * 48, [[16, 3], [1, 16]])
                nc.sync.dma_start(out_ap, in_ap)

        for p in (psm, ps, nrm, pq, stg, sml, big, con):
            p.release()

    nc.compile()
    return nc


def _host_consts(w1, w2, w3f_, w4f_):
    u1 = (w1 * w1).astype(np.float32)
    u2 = (w2 * w2).astype(np.float32)
    u3 = (w3f_ * w3f_).astype(np.float32)
    u4 = (w4f_ * w4f_).astype(np.float32)
    ones = np.ones((H,), np.float32)
    wsb = np.stack([ones, u4[0], u4[1], u2[0], u2[1],
                    u1[0], u1[1], u3[0], u3[1]], axis=1)  # (H, 9)
    return u1, u2, u3, u4, wsb


def _prepare(p_f, p_b, q_f, q_b, w1f, w1b, w2f, w2b, w3f, w3b, w4f, w4b):
    import ml_dtypes

    bfd = ml_dtypes.bfloat16
    if "prog" not in _COMPILED:
        _COMPILED["prog"] = _build_program()
    nc = _COMPILED["prog"]

    p_f, p_b = np.asarray(p_f), np.asarray(p_b)
    q_f, q_b = np.asarray(q_f), np.asarray(q_b)
    u1f, u2f_, u3, u4, wsbf = _host_consts(
        np.asarray(w1f), np.asarray(w2f), np.asarray(w3f), np.asarray(w4f))
    u1b, u2b_, _, _, wsbb = _host_consts(
        np.asarray(w1b), np.asarray(w2b), np.asarray(w3f), np.asarray(w4f))
    WSB = np.concatenate([wsbf, wsbb], axis=1).astype(np.float32)
    U1C = np.stack([u1f[0], u1f[1], u1b[0], u1b[1]], 1).astype(np.float32)
    U2C = np.stack([u2f_[0], u2f_[1], u2b_[0], u2b_[1]], 1).astype(np.float32)
    ubc = np.concatenate([u3[0], u3[1], u4[0], u4[1]]).astype(np.float32)
    UBC16 = np.ascontiguousarray(
        np.broadcast_to(ubc, (128, 400))).astype(bfd)
    SEL16 = np.zeros((9, 384), np.float32)
    SEL16[3, 128:256] = 1.0  # invq2_0
    SEL16[4, 256:384] = 1.0  # invq2_1
    IDN = np.eye(128, dtype=np.float32)

    in_maps = []
    for c in range(NCORES):
        sl = slice(c * BC, (c + 1) * BC)
        in_maps.append({
            "p_f": np.ascontiguousarray(p_f[:, sl, :]),
            "p_b": np.ascontiguousarray(p_b[:, sl, :]),
            "q_f": np.ascontiguousarray(q_f[:, sl, :]),
            "q_b": np.ascontiguousarray(q_b[:, sl, :]),
            "WSB16": WSB.astype(bfd), "WSBF": WSB,
            "U1C": U1C, "U2C": U2C, "UBC16": UBC16,
            "SEL16": SEL16.astype(bfd),
            "IDN": IDN, "IDN16": IDN.astype(bfd),
        })

    return nc, in_maps


def _gather_outputs(results):
    full = []
    for nm in ("m1f", "m1b", "m2f", "m2b", "m3f", "m3b", "m4f", "m4b"):
        full.append(np.concatenate([results[c][nm] for c in range(NCORES)],
                                   axis=1))
    return tuple(full)


def kernel(**inputs):
    from concourse.bass_utils import run_bass_kernel_spmd

    nc, in_maps = _prepare(**inputs)
    res = run_bass_kernel_spmd(nc, in_maps, list(range(NCORES)))
    return _gather_outputs(res.results)


def run_traced(**inputs):
    """Run with NTFF profiling; returns (exec_time_ns, results_obj)."""
    from concourse.bass_utils import run_bass_kernel_spmd

    nc, in_maps = _prepare(**inputs)
    res = run_bass_kernel_spmd(nc, in_maps, list(range(NCORES)), trace=True)
    return res.exec_time_ns, res
